# revision 54
# baseline (speedup 1.0000x reference)
"""Trainium2 Bass kernel for DocumentGraphEncoder (3-layer GATv2 + LN + gated pooling).

Self-contained: takes FULL inputs, shards across 8 NeuronCores internally,
returns FULL [64, 256] float32 output.

Sharding: nodes partitioned contiguously across 8 cores (3750/core, padded to
3840 = 32 groups of 120 dst nodes). Each core owns the edges whose dst is in
its range, LPT-balanced into groups and sorted by (dst_group, dst, src).

Per layer: the xl table (node-major fp16) is AllGathered across cores;
per-edge source rows arrive via 4-queue SWDGE dma_gather. Per 512-edge tile,
z = xl[src]+xr[dst]+ea@we is accumulated in PSUM feature-major: one folded
matmul does the xr broadcast AND the edge-attr projection (the one-hot rhs
carries ea values on rows 120-127, the lhsT carries we rows there), plus 4 PE
transposes of the gathered rows. LeakyReLU is a single Prelu activation op,
logits/segment-softmax/scatter are PE matmuls, the alpha multiply is one DVE
broadcast op. The next layer's xl dense is computed inline per group during
the edge phase so each AllGather fires immediately at edge-phase end (the xr
dense runs inside the collective's shadow; DMA-transposes are avoided since
the tile scheduler serializes them with collectives). Layer 3 stashes pre-LN
centred features; LN rstd/gate-exp run as a two-part batched tail (avoids
Sqrt/Exp act-table thrash) that pools cent directly; the LN affine + transform
fold into the tiny [64, 257] post-AllReduce stage.
"""
import numpy as np
from contextlib import ExitStack

import concourse.bass as bass
import concourse.bacc as bacc
import concourse.tile as tile
import concourse.mybir as mybir
from concourse._compat import get_trn_type, cdiv
from concourse.bass_utils import run_bass_kernel_spmd

FP16 = mybir.dt.float16
F32 = mybir.dt.float32
I16 = mybir.dt.int16
AFT = mybir.ActivationFunctionType
ALU = mybir.AluOpType

N, E, IN, HID, G = 30000, 480000, 399, 256, 64
NEG = 0.2
NCORE = 8
NLOC = N // NCORE          # 3750
GP = 120                   # dst nodes per group; rows 120-127 of the one-hot
                           # matmul carry the edge-attr projection (we-fold)
NGRP = cdiv(NLOC, GP)      # 32
NLOCP = NGRP * GP          # 3840
NP = NCORE * NLOCP         # 30720
KB1 = 4                    # 512 = padded IN contraction blocks
HEADS = (8, 8, 1)
LN_EPS = 1e-5
DEN_EPS = 1e-30

# knobs for compile-scaling experiments (full problem: 3, NGRP)
import os as _os
N_LAYERS = int(_os.environ.get("K_LAYERS", "3"))
NGRP_USE = int(_os.environ.get("K_NGRP", str(NGRP)))

_prog_cache = {}


def _wrap_idx(idx, egrp):
    """[..., EGRP] int16 -> wrapped [. , 128, EGRP//16] layout for dma_gather."""
    lead = idx.shape[:-1]
    w = np.zeros(lead + (128, egrp // 16), np.int16)
    r = idx.reshape(lead + (egrp // 16, 16))
    for rep in range(8):
        w[..., rep * 16:(rep + 1) * 16, :] = np.swapaxes(r, -1, -2)
    return w


def _host_prep(inputs):
    x = np.asarray(inputs["x"], np.float32)
    edge_index = np.asarray(inputs["edge_index"], np.int64)
    edge_attr = np.asarray(inputs["edge_attr"], np.float32)
    batch = np.asarray(inputs["batch"], np.int64)
    src, dst = edge_index[0], edge_index[1]

    import heapq
    core_of = dst // NLOC
    per_core = []
    perms = []
    maxgrp = 0
    for c in range(NCORE):
        m = np.nonzero(core_of == c)[0]
        ld0 = dst[m] - c * NLOC
        deg = np.bincount(ld0, minlength=NLOC)
        # LPT: assign nodes (desc degree) to least-loaded group with space
        order_n = np.argsort(-deg, kind="stable")
        heap = [(0, 0, gi) for gi in range(NGRP)]
        heapq.heapify(heap)
        perm = np.empty(NLOC, np.int64)
        for node in order_n:
            load, fill, gi = heapq.heappop(heap)
            perm[node] = gi * GP + fill
            if fill + 1 < GP:
                heapq.heappush(heap, (load + int(deg[node]), fill + 1, gi))
        perms.append(perm)
        ld = perm[ld0]
        g = ld // GP
        order = np.lexsort((src[m], ld))
        m, s, ld, g = m[order], src[m][order], ld[order], g[order]
        cnt = np.bincount(g, minlength=NGRP)
        maxgrp = max(maxgrp, int(cnt.max()))
        per_core.append((m, s, ld, g, cnt))
    egrp = cdiv(maxgrp, 512) * 512
    nchk, ntil = egrp // 128, egrp // 512

    all_perm = np.stack(perms)
    # per-core edge-order arrays; group gg processes nchk_gs[gg] 128-edge chunks
    gmax = np.zeros(NGRP, np.int64)
    for c in range(NCORE):
        gmax = np.maximum(gmax, per_core[c][4])
    nchk_gs = [int(cdiv(int(v), 128)) for v in gmax]
    host = {"egrp": egrp, "nchk": nchk, "ntil": ntil, "nchk_gs": nchk_gs, "cores": []}
    for c in range(NCORE):
        m, s, ld, g, cnt = per_core[c]
        src_pad = np.zeros((NGRP, egrp), np.int64)
        oh_em = np.zeros((NGRP, 128, nchk, GP), np.float16)
        oh_nm = np.zeros((NGRP, 128, ntil, 512), np.float16)
        off = np.concatenate([[0], np.cumsum(cnt)])
        for gg in range(NGRP):
            n_e = int(cnt[gg])
            sl = slice(off[gg], off[gg] + n_e)
            sg, ldg, mg = s[sl], ld[sl], m[sl]
            sc = sg // NLOC
            src_pad[gg, :n_e] = NLOCP * sc + all_perm[sc, sg % NLOC]
            rel = (ldg - gg * GP).astype(np.int64)
            ee = np.arange(n_e)
            oh_em[gg, ee % 128, ee // 128, rel] = 1.0
            oh_nm[gg, rel, ee // 512, ee % 512] = 1.0
            # rows 120-124: edge-attr values + bias-const row (we-fold)
            ea4 = edge_attr[mg].T.astype(np.float16)
            for j in range(4):
                oh_nm[gg, GP + j, ee // 512, ee % 512] = ea4[j]
            oh_nm[gg, GP + 4, ee // 512, ee % 512] = 1.0
        pc = all_perm[c]
        xs = np.zeros((NLOCP, 512), np.float32)
        xs[pc, :IN] = x[c * NLOC:(c + 1) * NLOC]
        bo = np.zeros((NGRP, 128, G), np.float16)
        bo[pc // GP, pc % GP, batch[c * NLOC:(c + 1) * NLOC]] = 1.0
        host["cores"].append({
            "xT": np.ascontiguousarray(xs.T).astype(np.float16),
            "src_idx": _wrap_idx(src_pad.astype(np.int16), egrp),
            "oh_em": oh_em,
            "oh_nm": oh_nm,
            "bonehot": bo,
        })

    # weights
    def f16(a):
        return np.asarray(a, np.float32).astype(np.float16)

    wmeta = {}
    dims = [(IN, 8, 32), (HID, 8, 32), (HID, 1, 256)]
    for li, (fin, h, cdim) in enumerate(dims, 1):
        kb = KB1 if li == 1 else 2
        wl = np.zeros((kb * 128, 256), np.float32)
        wr = np.zeros((kb * 128, 256), np.float32)
        wl[:fin] = np.asarray(inputs[f"wl{li}"], np.float32)
        wr[:fin] = np.asarray(inputs[f"wr{li}"], np.float32)
        wblk = np.zeros((2, kb, 2, 128, 128), np.float16)
        for t, w in enumerate((wl, wr)):
            for k in range(kb):
                for ob in range(2):
                    wblk[t, k, ob] = f16(w[k * 128:(k + 1) * 128, ob * 128:(ob + 1) * 128])
        we = np.asarray(inputs[f"we{li}"], np.float32)
        bl = np.asarray(inputs[f"bl{li}"], np.float32)
        br = np.asarray(inputs[f"br{li}"], np.float32)
        we_aug = np.zeros((8, 256), np.float16)
        we_aug[:4] = f16(we)
        we_aug[4] = f16(bl + br)
        att = np.asarray(inputs[f"att{li}"], np.float32)  # [h, cdim]
        blk = np.zeros((256, 8), np.float32)
        for hh in range(h):
            blk[hh * cdim:(hh + 1) * cdim, hh] = att[hh]
        attz = np.stack([f16(blk[:128]), f16(blk[128:])])
        atta = np.stack([f16(0.4 * blk[:128]), f16(0.4 * blk[128:])])
        nbias = np.tile((np.asarray(inputs[f"b{li}"], np.float32)
                         + bl).astype(np.float16), (128, 1))
        wblk_flat = np.ascontiguousarray(
            wblk.transpose(3, 0, 1, 2, 4).reshape(128, 2 * kb * 2 * 128))
        we_aug32 = np.ascontiguousarray(np.tile(we_aug[:, None, :], (1, NGRP, 1)))
        wmeta[li] = dict(kb=kb, h=h, wblk=wblk_flat, we_aug=we_aug32, attz=attz,
                         atta=atta, nbias=nbias)
        if li > 1:
            # wl rows for inline dense in the previous edge phase: [p, k, fout]
            wmeta[li]["wlrows"] = np.ascontiguousarray(
                f16(wl[:256]).reshape(2, 128, 256).transpose(1, 0, 2))

    consts = {
        "id128": np.eye(128, dtype=np.float16),
        "id8": np.eye(8, dtype=np.float16),
        "id64": np.eye(64, dtype=np.float32),
        "epsden": np.full((128, 1), DEN_EPS, np.float32),
        "epsln": np.full((128, 1), LN_EPS, np.float32),
        "gw2": np.tile(np.asarray(inputs["ln_w"], np.float32)
                       * np.asarray(inputs["gate_w"], np.float32)[:, 0], (128, 1)),
        "lnw64": np.tile(np.asarray(inputs["ln_w"], np.float32), (G, 1)),
        "lnb64": np.tile(np.asarray(inputs["ln_b"], np.float32), (G, 1)),
        "gateb": np.full((128, 1), float(np.asarray(inputs["gate_b"])[0])
                 + float(np.sum(np.asarray(inputs["ln_b"], np.float32)
                                * np.asarray(inputs["gate_w"], np.float32)[:, 0])),
                 np.float32),
        "trw": np.stack([np.asarray(inputs["tr_w"], np.float32)[:128],
                         np.asarray(inputs["tr_w"], np.float32)[128:]]),
        "trb": np.tile(np.asarray(inputs["tr_b"], np.float32), (64, 1)),
    }
    host["wmeta"] = wmeta
    host["consts"] = consts
    return host


def _build_program(egrp, nchk, ntil, wmeta_shapes, nchk_gs):
    nc = bacc.Bacc(get_trn_type() or "TRN2", target_bir_lowering=False,
                   debug=False, num_swdge_queues=4)

    # ---- external inputs ----
    xT_in = nc.dram_tensor("xT", [512, NLOCP], FP16, kind="ExternalInput")
    sidx_in = nc.dram_tensor("src_idx", [NGRP, 128, egrp // 16], I16, kind="ExternalInput")
    ohem_in = nc.dram_tensor("oh_em", [NGRP, 128, nchk, GP], FP16, kind="ExternalInput")
    ohnm_in = nc.dram_tensor("oh_nm", [NGRP, 128, ntil, 512], FP16, kind="ExternalInput")
    bo_in = nc.dram_tensor("bonehot", [NGRP, 128, G], FP16, kind="ExternalInput")
    w_in = {}
    for li in (1, 2, 3):
        kb = wmeta_shapes[li]
        w_in[li] = dict(
            wblk=nc.dram_tensor(f"wblk{li}", [128, 2 * kb * 2 * 128], FP16, kind="ExternalInput"),
            we_aug=nc.dram_tensor(f"we_aug{li}", [8, NGRP, 256], FP16, kind="ExternalInput"),
            attz=nc.dram_tensor(f"attz{li}", [2, 128, 8], FP16, kind="ExternalInput"),
            atta=nc.dram_tensor(f"atta{li}", [2, 128, 8], FP16, kind="ExternalInput"),
            nbias=nc.dram_tensor(f"nbias{li}", [128, 256], FP16, kind="ExternalInput"),
        )
        if li > 1:
            w_in[li]["wlrows"] = nc.dram_tensor(
                f"wlrows{li}", [128, 2, 256], FP16, kind="ExternalInput")
    _NOPRELOAD = ("trw",)
    cin = {k: nc.dram_tensor(k, list(v.shape),
                             FP16 if v.dtype == np.float16 else F32,
                             kind="ExternalInput")
           for k, v in {
               "id128": np.zeros((128, 128), np.float16),
               "id8": np.zeros((8, 8), np.float16),
               "id64": np.zeros((64, 64), np.float32),
               "epsden": np.zeros((128, 1), np.float32),
               "epsln": np.zeros((128, 1), np.float32),
               "gw2": np.zeros((128, 256), np.float32),
               "lnw64": np.zeros((G, 256), np.float32),
               "lnb64": np.zeros((G, 256), np.float32),
               "gateb": np.zeros((128, 1), np.float32),
               "trw": np.zeros((2, 128, 256), np.float32),
               "trb": np.zeros((64, 256), np.float32),
           }.items()}
    out_t = nc.dram_tensor("out", [G, HID], F32, kind="ExternalOutput")
    DBG = _os.environ.get("K_DEBUG", "0") == "1"
    ABL = _os.environ.get("K_ABL", "")
    if DBG:
        dbg_xl = nc.dram_tensor("dbg_xl", [NGRP, 128, 256], FP16, kind="ExternalOutput")
        dbg_xr = nc.dram_tensor("dbg_xr", [NGRP, 128, 256], FP16, kind="ExternalOutput")
        dbg_h = nc.dram_tensor("dbg_h", [NGRP, 128, 256], FP16, kind="ExternalOutput")
        dbg_xg = nc.dram_tensor("dbg_xg", [128, 0 + 1 * (512 // 128), 256], FP16, kind="ExternalOutput")
        dbg_z = nc.dram_tensor("dbg_z", [128, 512], FP16, kind="ExternalOutput")
        dbg_l = nc.dram_tensor("dbg_l", [8, 512], F32, kind="ExternalOutput")
        dbg_xlT = nc.dram_tensor("dbg_xlT", [128, 2, NLOCP], FP16, kind="ExternalOutput")
        dbg_msg = nc.dram_tensor("dbg_msg", [128, 4, 264], FP16, kind="ExternalOutput")
        dbg_acc = nc.dram_tensor("dbg_acc", [128, 264], F32, kind="ExternalOutput")
        dbg_hf = nc.dram_tensor("dbg_hf", [128, 256], FP16, kind="ExternalOutput")
        dbg_hall = nc.dram_tensor("dbg_hall", [NGRP, 128, 256], FP16, kind="ExternalOutput")
        dbg_hT2 = nc.dram_tensor("dbg_hT2", [128, 2, NLOCP], FP16, kind="ExternalOutput")
        dbg_xlT2 = nc.dram_tensor("dbg_xlT2", [128, 2, NLOCP], FP16, kind="ExternalOutput")
        dbg_pre = nc.dram_tensor("dbg_pre", [G, 257], F32, kind="ExternalOutput")
        dbg_lnh = nc.dram_tensor("dbg_lnh", [NGRP, 128, 256], FP16, kind="ExternalOutput")

    RG = [list(range(NCORE))]

    with tile.TileContext(nc) as tc, ExitStack() as octx:
        dram = octx.enter_context(tc.tile_pool(name="dram", bufs=1, space="DRAM"))
        xl_loc = dram.tile([NLOCP, 256], FP16)
        xl_fulls = [dram.tile([NP, 256], FP16, addr_space="Shared", name=f"xl_full{i}")
                    for i in range(3)]
        hT_d = dram.tile([128, 2, NLOCP], FP16)
        pre_in_d = dram.tile([G, 257], F32)
        pre_out_d = dram.tile([G, 257], F32, addr_space="Shared")

        cpool = octx.enter_context(tc.tile_pool(name="const", bufs=1))
        csb = {}
        for k, t in cin.items():
            if k in _NOPRELOAD:
                continue
            csb[k] = cpool.tile(list(t.shape), t.dtype, name=f"c_{k}")
            nc.sync.dma_start(csb[k][:], t[:])
        bo_sb = cpool.tile([128, NGRP, G], FP16)
        nc.sync.dma_start(bo_sb[:], bo_in[:].rearrange("g p b -> p g b"))

        persist = octx.enter_context(tc.tile_pool(name="persist", bufs=1))
        xr_nm = persist.tile([128, NGRP, 256], FP16)
        cent_all = persist.tile([128, NGRP, 256], FP16)
        ssq_all = persist.tile([128, NGRP], F32)
        gshat_all = persist.tile([128, NGRP], F32)
        pre_acc = persist.tile([G, 257], F32)
        nc.vector.memset(pre_acc[:], 0.0)

        for li in range(1, N_LAYERS + 1):
            kb = wmeta_shapes[li]
            hh = HEADS[li - 1]
            wt = w_in[li]

            # ================= dense phase =================
            with ExitStack() as lctx:
                dp = lctx.enter_context(tc.tile_pool(name=f"d{li}", bufs=1))
                dps = lctx.enter_context(tc.tile_pool(name=f"dps{li}", bufs=2, space="PSUM"))
                dnm = lctx.enter_context(tc.tile_pool(name=f"dnm{li}", bufs=2, space="PSUM"))
                stg = lctx.enter_context(tc.tile_pool(name=f"stg{li}", bufs=3))

                w_sb = dp.tile([128, 2 * kb * 2 * 128], FP16)
                nc.sync.dma_start(w_sb[:], wt["wblk"][:])

                def wslice(t, k, ob):
                    base = ((t * kb + k) * 2 + ob) * 128
                    return w_sb[:, base:base + 128]

                xlT = dp.tile([128, 2, NLOCP], FP16, name="xlT") if li == 1 else None
                xrT = dp.tile([128, 2, NLOCP], FP16)
                NT = 480
                hT = dp.tile([128, kb, NLOCP], FP16)

                def dense_t(t, dst_t):
                    for ob in range(2):
                        for nt in range(NLOCP // NT):
                            ps = dps.tile([128, NT], F32, name="ps_dense")
                            for k in range(kb):
                                nc.tensor.matmul(ps[:], wslice(t, k, ob),
                                                 hT[:, k, nt * NT:(nt + 1) * NT],
                                                 start=(k == 0), stop=(k == kb - 1))
                            nc.vector.tensor_copy(dst_t[:, ob, nt * NT:(nt + 1) * NT], ps[:])

                if li == 1:
                    # xl first: dense -> node-major -> DRAM -> AllGather ASAP
                    for k in range(kb):
                        nc.sync.dma_start(hT[:, k, :], xT_in[k * 128:(k + 1) * 128, :])
                    dense_t(0, xlT)
                    for gg in range(NGRP):
                        psn = dnm.tile([128, 256], FP16, name="ps_nm")
                        for ob in range(2):
                            nc.tensor.transpose(psn[:GP, ob * 128:(ob + 1) * 128],
                                                xlT[:, ob, gg * GP:(gg + 1) * GP],
                                                csb["id128"][:])
                        st = stg.tile([128, 256], FP16, name="st_nm")
                        nc.scalar.activation(st[:GP, :], psn[:GP, :], AFT.Copy)
                        nc.sync.dma_start(xl_loc[gg * GP:(gg + 1) * GP, :], st[:GP, :])

                if ABL != "noag":
                    # li>1: xl_loc was written incrementally during the previous
                    # edge phase, so the collective fires immediately.
                    nc.gpsimd.collective_compute(
                        "AllGather", ALU.bypass, replica_groups=RG,
                        ins=[xl_loc[:].opt()], outs=[xl_fulls[li - 1][:].opt()])
                if li == 1:
                    # pad partitions (GP..127) are never written by per-group
                    # compute; zero them (during the collective) so the layer-3
                    # pooling matmuls read finite values
                    nc.vector.memset(cent_all[:], 0.0)
                    nc.vector.memset(ssq_all[:], 0.0)
                    nc.vector.memset(gshat_all[:], 0.0)

                # xr during the collective
                if li > 1:
                    nc.sync.dma_start(hT[:], hT_d[:])
                dense_t(1, xrT)
                for gg in range(NGRP):
                    for ob in range(2):
                        psn2 = dnm.tile([128, 128], FP16, name="ps_nm2")
                        nc.tensor.transpose(psn2[:GP, :],
                                            xrT[:, ob, gg * GP:(gg + 1) * GP],
                                            csb["id128"][:])
                        nc.vector.tensor_copy(xr_nm[:GP, gg, ob * 128:(ob + 1) * 128],
                                              psn2[:GP, :])
                if DBG and li == 1:
                    nc.sync.dma_start(dbg_xlT[:], xlT[:])

            # ================= edge phase =================
            with ExitStack() as lctx:
                ep = lctx.enter_context(tc.tile_pool(name=f"e{li}", bufs=3))
                gbuf = lctx.enter_context(tc.tile_pool(name=f"g{li}", bufs=5))
                epz = lctx.enter_context(tc.tile_pool(name=f"ez{li}", bufs=2, space="PSUM"))
                epl = lctx.enter_context(tc.tile_pool(name=f"el{li}", bufs=1, space="PSUM"))
                epe = lctx.enter_context(tc.tile_pool(name=f"ee{li}", bufs=1, space="PSUM"))
                epp = lctx.enter_context(tc.tile_pool(name=f"ep{li}", bufs=1, space="PSUM"))
                epa = lctx.enter_context(tc.tile_pool(name=f"ea{li}", bufs=2, space="PSUM"))
                epi = lctx.enter_context(tc.tile_pool(name=f"ei{li}", bufs=1, space="PSUM"))
                wp = lctx.enter_context(tc.tile_pool(name=f"w{li}", bufs=1))

                # we rows (edge-attr projection + bias) live at partitions
                # 120-127 of every group's xr slab; one DMA fills all groups
                nc.sync.dma_start(xr_nm[GP:128, :, :], wt["we_aug"][:])
                if li < 3:
                    wlr_sb = wp.tile([128, 2, 256], FP16)
                    nc.sync.dma_start(wlr_sb[:], w_in[li + 1]["wlrows"][:])
                attz_sb = wp.tile([128, 2, 8], FP16)
                nc.sync.dma_start(attz_sb[:], wt["attz"][:].rearrange("f p h -> p f h"))
                nbias_sb = wp.tile([128, 256], FP16)
                nc.sync.dma_start(nbias_sb[:], wt["nbias"][:])

                def ln_tail(g0, g1):
                    # batched LN/gate tail over groups [g0, g1): isolates the
                    # Sqrt/Exp act-table switch from the per-tile Exp stream
                    if g1 <= g0:
                        return
                    ng = g1 - g0
                    sd_all = wp.tile([128, NGRP], F32, name="sd_all")
                    nc.scalar.activation(sd_all[:, :ng], ssq_all[:, g0:g1], AFT.Sqrt,
                                         scale=1.0 / 256.0, bias=csb["epsln"][:])
                    rstd_all = wp.tile([128, NGRP], F32, name="rstd_all")
                    nc.vector.reciprocal(rstd_all[:, :ng], sd_all[:, :ng])
                    gs_all = wp.tile([128, NGRP], F32, name="gs_all")
                    nc.vector.tensor_mul(gs_all[:, :ng], gshat_all[:, g0:g1],
                                         rstd_all[:, :ng])
                    eg_all = wp.tile([128, NGRP], F32, name="eg_all")
                    nc.scalar.activation(eg_all[:, :ng], gs_all[:, :ng], AFT.Exp,
                                         bias=csb["gateb"][:])
                    eg16_all = wp.tile([128, NGRP], FP16, name="eg16_all")
                    nc.vector.tensor_copy(eg16_all[:, :ng], eg_all[:, :ng])
                    w2_all = wp.tile([128, NGRP], FP16, name="w2_all")
                    nc.vector.tensor_mul(w2_all[:, :ng], eg_all[:, :ng],
                                         rstd_all[:, :ng])
                    for gg in range(g0, g1):
                        wg2 = ep.tile([128, G], FP16, name="wg2")
                        nc.vector.tensor_mul(
                            wg2[:], bo_sb[:, gg, :],
                            w2_all[:, gg - g0:gg - g0 + 1].broadcast_to([128, G]))
                        psp = epp.tile([G, 257], F32, name="psp")
                        nc.tensor.matmul(psp[:, :256], wg2[:], cent_all[:, gg, :],
                                         start=True, stop=True)
                        nc.tensor.matmul(psp[:, 256:257], bo_sb[:, gg, :],
                                         eg16_all[:, gg - g0:gg - g0 + 1],
                                         start=True, stop=True)
                        nc.vector.tensor_add(pre_acc[:], pre_acc[:], psp[:])

                for gg in range(NGRP_USE):
                    nch_g = nchk_gs[gg]
                    eg_g = nch_g * 128
                    idx_sb = gbuf.tile([128, egrp // 16], I16, name="idx")
                    nc.sync.dma_start(idx_sb[:, :eg_g // 16], sidx_in[gg, :, :eg_g // 16])
                    ohe_sb = ep.tile([128, nchk, GP], FP16, name="ohe")
                    nc.sync.dma_start(ohe_sb[:, :nch_g, :], ohem_in[gg, :, :nch_g, :])
                    ohn_sb = ep.tile([128, ntil * 512], FP16, name="ohn")
                    nc.sync.dma_start(
                        ohn_sb[:, :eg_g],
                        ohnm_in[gg].rearrange("p t w -> p (t w)")[:, :eg_g])
                    xg = gbuf.tile([128, nchk, 256], FP16, name="xg")
                    if ABL != "nogather":
                        nc.gpsimd.dma_gather(xg[:, :nch_g, :], xl_fulls[li - 1][:],
                                             idx_sb[:, :eg_g // 16], eg_g, eg_g,
                                             256, single_packet=False, queue_num=gg % 4)
                    else:
                        nc.vector.memset(xg[:, 0, :], 0.25)
                        nc.vector.memset(xg[:, nch_g - 1, :], 0.25)

                    acc = epa.tile([128, 264], F32, name="acc")
                    if ABL == "nogather":
                        for cc in range(1, nch_g - 1):
                            nc.vector.memset(xg[:, cc, :], 0.25)
                    t_off = 0
                    while t_off < nch_g:
                        # a "pair" of up to 8 chunks shares one pl/expT/msg
                        pch = min(4, nch_g - t_off)
                        pl = epl.tile([hh, 512], F32, name="pl")
                        so = 0
                        while so < pch:
                            nch = min(4, pch - so)
                            W = nch * 128
                            co = (t_off + so) * 128
                            for fb in range(2):
                                pz = epz.tile([128, 512], F32, name="pz")
                                nc.tensor.matmul(pz[:, :W],
                                                 xr_nm[:, gg, fb * 128:(fb + 1) * 128],
                                                 ohn_sb[:, co:co + W],
                                                 start=True, stop=False)
                                for c4 in range(nch):
                                    nc.tensor.matmul(pz[:, c4 * 128:(c4 + 1) * 128],
                                                     xg[:, t_off + so + c4,
                                                        fb * 128:(fb + 1) * 128],
                                                     csb["id128"][:], start=False,
                                                     stop=(c4 == nch - 1))
                                uT = ep.tile([128, 512], FP16, name="uT")
                                nc.scalar.activation(uT[:, :W], pz[:, :W], AFT.Prelu,
                                                     alpha=NEG)
                                nc.tensor.matmul(pl[:, so * 128:so * 128 + W],
                                                 attz_sb[:, fb, :hh], uT[:, :W],
                                                 start=(fb == 0), stop=(fb == 1))
                            so += nch
                        expT = ep.tile([hh, 512], FP16, name="expT")
                        nc.scalar.activation(expT[:, :pch * 128], pl[:, :pch * 128],
                                             AFT.Exp)
                        pse = epe.tile([128, 4, 8], F32, name="pse")
                        for c4 in range(pch):
                            nc.tensor.matmul(pse[:, c4, :hh],
                                             expT[:, c4 * 128:(c4 + 1) * 128],
                                             csb["id8"][:hh, :hh], start=True, stop=True)
                        msg = ep.tile([128, 4, 264], FP16, name="msg")
                        nc.vector.tensor_copy(msg[:, :pch, 256:256 + hh],
                                              pse[:, :pch, :hh])
                        if hh == 8:
                            ebc = msg[:, :pch, 256:264][:, :, :, None] \
                                .broadcast_to([128, pch, 8, 32])
                        else:
                            ebc = msg[:, :pch, 256:257][:, :, :, None] \
                                .broadcast_to([128, pch, 1, 256])
                        nc.vector.tensor_mul(
                            msg[:, :pch, :256].rearrange("p c (h w) -> p c h w", h=hh),
                            xg[:, t_off:t_off + pch, :].rearrange(
                                "p a (h w) -> p a h w", h=hh),
                            ebc)
                        for c4 in range(pch):
                            nc.tensor.matmul(acc[:GP, :], ohe_sb[:, t_off + c4, :],
                                             msg[:, c4, :],
                                             start=(t_off == 0 and c4 == 0),
                                             stop=(t_off + pch == nch_g and c4 == pch - 1))
                        t_off += pch

                    # -------- normalize group --------
                    if DBG and li == 1 and gg == 0:
                        accst = ep.tile([128, 264], F32, name="accst")
                        nc.scalar.activation(accst[:], acc[:], AFT.Copy)
                        nc.sync.dma_start(dbg_acc[:], accst[:])
                    den = ep.tile([128, 8], F32, name="den")
                    nc.vector.tensor_scalar_add(den[:GP, :hh], acc[:GP, 256:256 + hh],
                                                DEN_EPS)
                    rec = ep.tile([128, 8], F32, name="rec")
                    nc.vector.reciprocal(rec[:GP, :hh], den[:GP, :hh])
                    if li < 3:
                        h0 = ep.tile([128, 256], FP16, name="h0")
                        rbc = (rec[:GP, :hh][:, :, None]
                               .broadcast_to([GP, hh, 256 // hh]))
                        nc.vector.tensor_mul(
                            h0[:GP, :].rearrange("p (h w) -> p h w", h=hh),
                            acc[:GP, :256].rearrange("p (h w) -> p h w", h=hh), rbc)
                        hb = ep.tile([128, 256], FP16, name="hb")
                        nc.vector.tensor_add(hb[:GP, :], h0[:GP, :], nbias_sb[:GP, :])
                        # ELU: max(x,0) + exp(min(x,0)) - 1, with
                        # exp(min(x,0)) = min(exp(x), 1) (exp overflow -> inf -> 1)
                        r_ = ep.tile([128, 256], FP16, name="relu")
                        nc.vector.tensor_scalar_max(r_[:GP, :], hb[:GP, :], 0.0)
                        en = ep.tile([128, 256], FP16, name="expn")
                        nc.scalar.activation(en[:GP, :], hb[:GP, :], AFT.Exp)
                        em = ep.tile([128, 256], FP16, name="expm")
                        nc.vector.tensor_scalar(em[:GP, :], en[:GP, :], 1.0, -1.0,
                                                op0=ALU.min, op1=ALU.add)
                        hf = ep.tile([128, 256], FP16, name="hf")
                        nc.vector.tensor_add(hf[:GP, :], r_[:GP, :], em[:GP, :])
                        # inline next-layer xl for this group so the next
                        # AllGather can fire as soon as this edge phase ends;
                        # h is staged to DRAM feature-major for the xr dense
                        hfT_ps = epi.tile([128, 2, GP], FP16, name="hfT_ps")
                        for k in range(2):
                            nc.tensor.transpose(hfT_ps[:, k, :],
                                                hf[:GP, k * 128:(k + 1) * 128],
                                                csb["id128"][:GP, :GP])
                        hfT_sb = ep.tile([128, 2, GP], FP16, name="hfT_sb")
                        nc.vector.tensor_copy(hfT_sb[:], hfT_ps[:])
                        nc.sync.dma_start(hT_d[:, :, gg * GP:(gg + 1) * GP], hfT_sb[:])
                        xlnm_ps = epi.tile([128, 256], F32, name="xlnm_ps")
                        for k in range(2):
                            nc.tensor.matmul(xlnm_ps[:GP, :], hfT_sb[:, k, :],
                                             wlr_sb[:, k, :],
                                             start=(k == 0), stop=(k == 1))
                        xl_st = ep.tile([128, 256], FP16, name="xl_st")
                        nc.vector.tensor_copy(xl_st[:GP, :], xlnm_ps[:GP, :])
                        nc.sync.dma_start(xl_loc[gg * GP:(gg + 1) * GP, :], xl_st[:GP, :])
                    else:
                        # pre-LN features: cent = hb - mean(hb); stash cent,
                        # ssq, and the gate partial sum(cent * lnw * gatew).
                        # LN scale/shift and exp(gate) are applied in a batched
                        # tail so Sqrt/Exp never interleave on the Act queue.
                        h0 = ep.tile([128, 256], F32, name="h0f")
                        rbc = rec[:GP, :1][:, :, None].broadcast_to([GP, 1, 256])
                        nc.vector.tensor_mul(
                            h0[:GP, :].rearrange("p (h w) -> p h w", h=1),
                            acc[:GP, :256].rearrange("p (h w) -> p h w", h=1), rbc)
                        hb = ep.tile([128, 256], F32, name="hbf")
                        nc.vector.tensor_add(hb[:GP, :], h0[:GP, :], nbias_sb[:GP, :])
                        mu = ep.tile([128, 1], F32, name="mu")
                        nc.vector.reduce_sum(mu[:GP, :], hb[:GP, :],
                                             axis=mybir.AxisListType.X)
                        nmu = ep.tile([128, 1], F32, name="nmu")
                        nc.vector.tensor_scalar_mul(nmu[:GP, :], mu[:GP, :], -1.0 / 256.0)
                        cent = ep.tile([128, 256], F32, name="cent")
                        nc.vector.tensor_scalar_add(cent[:GP, :], hb[:GP, :], nmu[:GP, :])
                        sq = ep.tile([128, 256], F32, name="sq")
                        nc.scalar.activation(sq[:GP, :], cent[:GP, :], AFT.Square,
                                             accum_out=ssq_all[:GP, gg:gg + 1])
                        nc.vector.tensor_copy(cent_all[:GP, gg, :], cent[:GP, :])
                        gm = ep.tile([128, 256], F32, name="gm")
                        nc.vector.tensor_mul(gm[:GP, :], cent[:GP, :], csb["gw2"][:GP, :])
                        nc.vector.reduce_sum(gshat_all[:GP, gg:gg + 1], gm[:GP, :],
                                             axis=mybir.AxisListType.X)
                        if gg == 15:
                            ln_tail(0, 16)

                if li == 3:
                    ln_tail(16, NGRP_USE)

        # ================= final: allreduce + transform =================
        with ExitStack() as lctx:
            fp_ = lctx.enter_context(tc.tile_pool(name="fin", bufs=1))
            fps = lctx.enter_context(tc.tile_pool(name="finps", bufs=2, space="PSUM"))
            nc.sync.dma_start(pre_in_d[:], pre_acc[:])
            nc.gpsimd.collective_compute(
                "AllReduce", ALU.add, replica_groups=RG,
                ins=[pre_in_d[:].opt()], outs=[pre_out_d[:].opt()])
            pre_all = fp_.tile([G, 257], F32)
            nc.sync.dma_start(pre_all[:], pre_out_d[:])
            if DBG:
                nc.sync.dma_start(dbg_pre[:], pre_acc[:])
            deng = fp_.tile([G, 1], F32)
            nc.vector.tensor_copy(deng[:], pre_all[:, 256:257])
            recg = fp_.tile([G, 1], F32)
            nc.vector.reciprocal(recg[:], deng[:])
            # h_ln pooled = lnw * (S / sumw) + lnb  (LN affine applied here)
            pn = fp_.tile([G, 256], F32)
            nc.vector.tensor_scalar_mul(pn[:], pre_all[:, :256], recg[:])
            pn2 = fp_.tile([G, 256], F32)
            nc.vector.tensor_mul(pn2[:], pn[:], csb["lnw64"][:])
            pn3 = fp_.tile([G, 256], F32)
            nc.vector.tensor_add(pn3[:], pn2[:], csb["lnb64"][:])
            preT = fp_.tile([128, 2, G], F32)
            for fb in range(2):
                pst = fps.tile([128, G], F32, name="pst")
                nc.tensor.matmul(pst[:], pn3[:, fb * 128:(fb + 1) * 128],
                                 csb["id64"][:], start=True, stop=True)
                nc.vector.tensor_copy(preT[:, fb, :], pst[:])
            trw_sb = fp_.tile([128, 2, 256], F32)
            nc.sync.dma_start(trw_sb[:], cin["trw"][:].rearrange("f p m -> p f m"))
            pso = fps.tile([G, 256], F32, name="pso")
            for fb in range(2):
                nc.tensor.matmul(pso[:], preT[:, fb, :], trw_sb[:, fb, :],
                                 start=(fb == 0), stop=(fb == 1))
            outs = fp_.tile([G, 256], F32)
            nc.scalar.activation(outs[:], pso[:], AFT.Copy)
            outf = fp_.tile([G, 256], F32)
            nc.vector.tensor_add(outf[:], outs[:], csb["trb"][:])
            nc.sync.dma_start(out_t[:], outf[:])

    nc.compile()
    return nc


def build(inputs):
    host = _host_prep(inputs)
    egrp, nchk, ntil = host["egrp"], host["nchk"], host["ntil"]
    key = (egrp, N_LAYERS, NGRP_USE, tuple(host["nchk_gs"]), _os.environ.get("K_ABL", ""))
    if key not in _prog_cache:
        _prog_cache[key] = _build_program(egrp, nchk, ntil,
                                          {li: host["wmeta"][li]["kb"] for li in (1, 2, 3)},
                                          host["nchk_gs"])
    nc = _prog_cache[key]

    in_maps = []
    for c in range(NCORE):
        hc = host["cores"][c]
        m = {
            "xT": hc["xT"], "src_idx": hc["src_idx"],
            "oh_em": hc["oh_em"], "oh_nm": hc["oh_nm"], "bonehot": hc["bonehot"],
        }
        for li in (1, 2, 3):
            wm = host["wmeta"][li]
            m[f"wblk{li}"] = wm["wblk"]
            m[f"we_aug{li}"] = wm["we_aug"]
            m[f"attz{li}"] = np.ascontiguousarray(wm["attz"])
            m[f"atta{li}"] = np.ascontiguousarray(wm["atta"])
            m[f"nbias{li}"] = wm["nbias"]
            if li > 1:
                m[f"wlrows{li}"] = wm["wlrows"]
        for k, v in host["consts"].items():
            m[k] = np.ascontiguousarray(v)
        in_maps.append(m)
    return nc, in_maps


def kernel(**inputs):
    nc, in_maps = build(inputs)
    res = run_bass_kernel_spmd(nc, in_maps, list(range(NCORE)))
    return np.asarray(res.results[0]["out"], np.float32)

